# revision 28
# baseline (speedup 1.0000x reference)
"""Euclidean-distance attention on 8 Trainium2 NeuronCores.

Sharding: batch (2) x head-groups (4 heads each) -> 8 cores; each core
computes Q/K/V projections for its 4 heads (column-sliced weights), a
flash-style transposed-score attention, and a partial out-projection
(row-sliced wo). Host sums the 4 partials per batch (row-parallel out_proj
reduction) and adds the output bias.

Math trick: softmax_k(-max(||q||^2+||k||^2-2qk, 0)/T) == softmax_k((2qk-||k||^2)/T)
(the ||q||^2 term is constant per row and cancels; the max() clamp never fires
because d^2 >= 0 up to rounding).  With scores computed transposed
(scT[k, q] = K @ Q^T), the per-k bias -||k||^2/T is a per-partition vector and
folds into the scalar-engine exp activation: p~ = exp(scale*scT + bias).
Normalization uses an extra all-ones column appended to V, so the softmax
denominator falls out of the same PSUM accumulation as the numerator.
"""

import sys

sys.path.insert(0, "/opt/trn_rl_repo")

import numpy as np

import concourse.bass as bass
import concourse.tile as tile
from concourse import bacc, mybir
from concourse.bass_utils import run_bass_kernel_spmd
from concourse.masks import make_identity

F32 = mybir.dt.float32
BF16 = mybir.dt.bfloat16

E = 1024          # embed dim
D = 64            # head dim
HLOC = 4          # heads per core
DH = HLOC * D     # 256: per-core projection width
P = 128
N_CORES = 8


def build_program(S, temperature, zq, zk, zv):
    """Trace the per-core program. All 8 cores run this same program on
    different input slices. zq/zk/zv: bias-is-zero flags (skip the adds)."""
    T = float(temperature)
    NT = S // P           # token tiles
    NE = E // P           # embed (contraction) tiles
    NPR = HLOC // 2       # head pairs (2)
    QW = min(512, S)      # q block width for score matmuls
    NQB = S // QW         # q blocks
    GW = 2 * QW           # exp tile width (2 q-blocks share one ACT call)

    # Bacc (not raw Bass): its compile() passes legalize multi-wait
    # instructions (move_matmul_waits_to_ldweights, generate_event_semaphores)
    # for the 1-wait-per-instruction hardware encoding.
    nc = bacc.Bacc(None)
    x_d = nc.dram_tensor("x", [S, E], F32, kind="ExternalInput")
    wq_d = nc.dram_tensor("wq_s", [E, DH], F32, kind="ExternalInput")
    wk_d = nc.dram_tensor("wk_s", [E, DH], F32, kind="ExternalInput")
    wv_d = nc.dram_tensor("wv_s", [E, DH], F32, kind="ExternalInput")
    wo_d = nc.dram_tensor("wo_s", [DH, E], F32, kind="ExternalInput")
    bq_d = nc.dram_tensor("bq_s", [DH], F32, kind="ExternalInput")
    bk_d = nc.dram_tensor("bk_s", [DH], F32, kind="ExternalInput")
    bv_d = nc.dram_tensor("bv_s", [DH], F32, kind="ExternalInput")
    # one output tensor per token tile, stored straight from PSUM: 16
    # stores spread across the 16 DMA queues (8 HWDGE + 8 SWDGE) so each
    # queue sees one store -> no same-queue ordering wait, leaving the
    # single descriptor wait slot for the RAW wait on the matmuls
    y_ds = [
        nc.dram_tensor(f"y{tt}", [P, E], F32, kind="ExternalOutput")
        for tt in range(S // P)
    ]

    def bcast_ap(ap_1d, parts):
        # [N] dram vector -> [parts, N] partition-broadcast AP
        return bass.AP(
            tensor=ap_1d.tensor, offset=ap_1d.offset, ap=[[0, parts]] + list(ap_1d.ap)
        )

    with tile.TileContext(nc) as tc:
        with tc.tile_pool(name="consts", bufs=1) as consts, \
             tc.tile_pool(name="big", bufs=1) as big, \
             tc.tile_pool(name="xbpool", bufs=3) as xbpool, \
             tc.tile_pool(name="pTpool", bufs=3) as pTpool, \
             tc.tile_pool(name="dbpool", bufs=2) as dbpool:
            # ---- constants / weights staging ----
            ident = consts.tile([P, P], BF16)
            make_identity(nc, ident)

            # weights: casting SWDGE DMAs (f32->bf16 in flight), chunked
            # per contraction tile so the 8 SWDGE queues run in parallel
            wq_sb = consts.tile([P, NE, DH], BF16)
            wk_sb = consts.tile([P, NE, DH], BF16)
            wv_sb = consts.tile([P, NE, DH], BF16)
            wo_sb = consts.tile([P, 2, E], BF16)

            # all-ones stationary for the denominator broadcast matmul;
            # row 64 (= base_partition of the denominator row) is what's used
            ones_col = consts.tile([P, D], F32)
            nc.vector.memset(ones_col, 1.0)

            bq_col = consts.tile([P, NPR], F32)
            nc.gpsimd.dma_start(bq_col, bq_d[:].rearrange("(pr p) -> p pr", p=P))
            bk_col = consts.tile([P, NPR], F32)
            nc.gpsimd.dma_start(bk_col, bk_d[:].rearrange("(pr p) -> p pr", p=P))
            bk_bc = consts.tile([P, DH], F32)
            nc.gpsimd.dma_start(bk_bc, bcast_ap(bk_d[:], P))
            bv_bc = consts.tile([P, DH], F32)
            nc.gpsimd.dma_start(bv_bc, bcast_ap(bv_d[:], P))

            # ---- persistent big tiles ----
            xb_all = big.tile([P, NT, E], BF16)    # query, cast to bf16
            qT = big.tile([P, NE, S], BF16)        # query^T (e-major)
            QT_sb = big.tile([P, NPR, S], BF16)    # Q^T per head-pair
            KT_sb = big.tile([P, NPR, S], BF16)
            V_sb = big.tile([P, NT, HLOC, D + 1], BF16)   # V + ones column
            nksq = big.tile([P, NT, HLOC], F32)    # -||k||^2 / T
            ou_all = big.tile([P, HLOC, NQB, QW], F32)  # unnormalized attn out
            aoT = big.tile([P, NPR, S], BF16)      # normalized attn out^T

            nc.gpsimd.memset(V_sb[:, :, :, D], 1.0)

            # first x tiles in flight before anything else queues on the
            # DMAs, split per e-chunk so the first transposes start early
            for j in range(QW // P):
                for e in range(NE):
                    nc.gpsimd.dma_start(
                        xb_all[:, j, e * P:(e + 1) * P],
                        x_d[j * P:(j + 1) * P, e * P:(e + 1) * P],
                    )
            for w_d, w_sb in ((wq_d, wq_sb), (wk_d, wk_sb), (wv_d, wv_sb)):
                wr = w_d[:].rearrange("(e p) d -> p e d", p=P)
                for e in range(NE):
                    nc.gpsimd.dma_start(w_sb[:, e, :], wr[:, e, :])
            wor = wo_d[:].rearrange("(s p) d -> p s d", p=P)
            for s2 in range(2):
                nc.gpsimd.dma_start(wo_sb[:, s2, :], wor[:, s2, :])

            # ---- phase 1+2: transpose x, projections ----
            with tc.tile_pool(name="ps_tr", bufs=2, space="PSUM") as ps_tr, \
                 tc.tile_pool(name="ps_pj", bufs=2, space="PSUM") as ps_pj, \
                 tc.tile_pool(name="ps_kv", bufs=2, space="PSUM") as ps_kv:
                for blk in range(S // QW):
                    jlo = blk * (QW // P)
                    jhi = jlo + QW // P
                    for j in range(jlo, jhi):
                        if j >= QW // P:  # first block's DMAs issued above
                            nc.gpsimd.dma_start(
                                xb_all[:, j, :], x_d[j * P:(j + 1) * P, :])
                        for e in range(NE):
                            pt = ps_tr.tile([P, P], BF16, tag="tr")
                            nc.tensor.transpose(pt, xb_all[:, j, e * P:(e + 1) * P], ident)
                            nc.vector.tensor_copy(qT[:, e, j * P:(j + 1) * P], pt)
                    bsl = slice(blk * QW, (blk + 1) * QW)
                    # Q^T and K^T per head pair over this token block
                    for pr in range(NPR):
                        psl = slice(pr * P, (pr + 1) * P)
                        for w_sb, dst, bias_col, bz in (
                            (wq_sb, QT_sb, bq_col[:, pr:pr + 1], zq),
                            (wk_sb, KT_sb, bk_col[:, pr:pr + 1], zk),
                        ):
                            pj = ps_pj.tile([P, QW], F32, tag="pj")
                            for e in range(NE):
                                nc.tensor.matmul(
                                    pj,
                                    lhsT=w_sb[:, e, psl],
                                    rhs=qT[:, e, bsl],
                                    start=(e == 0),
                                    stop=(e == NE - 1),
                                )
                            if bz:
                                nc.vector.tensor_copy(dst[:, pr, bsl], pj)
                            else:
                                # bias is per-partition (head-dim) in ^T layout
                                nc.vector.tensor_scalar_add(
                                    out=dst[:, pr, bsl], in0=pj, scalar1=bias_col
                                )
                    # V (token-major) and -||k||^2/T over this token block
                    for j in range(jlo, jhi):
                        pv = ps_kv.tile([P, DH], F32, tag="pv")
                        for e in range(NE):
                            nc.tensor.matmul(
                                pv,
                                lhsT=qT[:, e, j * P:(j + 1) * P],
                                rhs=wv_sb[:, e, :],
                                start=(e == 0),
                                stop=(e == NE - 1),
                            )
                        for h in range(HLOC):
                            if zv:
                                # scalar engine, so the attn*V matmul's waits
                                # on V and on the exp output share one sem
                                nc.scalar.copy(
                                    V_sb[:, j, h, 0:D], pv[:, h * D:(h + 1) * D]
                                )
                            else:
                                nc.vector.tensor_add(
                                    out=V_sb[:, j, h, 0:D],
                                    in0=pv[:, h * D:(h + 1) * D],
                                    in1=bv_bc[:, h * D:(h + 1) * D],
                                )
                        pk = ps_kv.tile([P, DH], F32, tag="pk")
                        for e in range(NE):
                            nc.tensor.matmul(
                                pk,
                                lhsT=qT[:, e, j * P:(j + 1) * P],
                                rhs=wk_sb[:, e, :],
                                start=(e == 0),
                                stop=(e == NE - 1),
                            )
                        sq_t = xbpool.tile([P, DH], F32, tag="sq")
                        if zk:
                            # only one non-scalar PSUM input allowed per DVE op
                            nc.vector.tensor_copy(sq_t, pk)
                        else:
                            nc.vector.tensor_add(out=sq_t, in0=pk, in1=bk_bc)
                        nc.vector.tensor_mul(sq_t, sq_t, sq_t)
                        ksqr = xbpool.tile([P, HLOC], F32, tag="ksqr")
                        nc.vector.tensor_reduce(
                            out=ksqr,
                            in_=sq_t.rearrange("p (h d) -> p h d", h=HLOC),
                            axis=mybir.AxisListType.X,
                            op=mybir.AluOpType.add,
                        )
                        nc.scalar.mul(nksq[:, j, :], ksqr, -1.0 / T)

            # ---- phase 3: attention ----
            with tc.tile_pool(name="ps_sc", bufs=2, space="PSUM") as ps_sc, \
                 tc.tile_pool(name="ps_av", bufs=1, space="PSUM") as ps_av, \
                 tc.tile_pool(name="ypool", bufs=4) as ypool:
                for h in range(HLOC):
                    pr = h // 2
                    off = (h % 2) * D
                    av_ts = [
                        ps_av.tile([P, QW], F32, tag=f"av{qb}", name=f"av{qb}")
                        for qb in range(NQB)
                    ]
                    if h == 0:
                        for qb in range(NQB):
                            nc.vector.memset(av_ts[qb], 0.0)
                    for j in range(NT):
                        for g0 in range(0, NQB, 2):
                            gn = min(2, NQB - g0)
                            sc_t = ps_sc.tile([P, gn * QW], F32, tag="sc")
                            for qq in range(gn):
                                qb = g0 + qq
                                nc.tensor.matmul(
                                    sc_t[:, qq * QW:(qq + 1) * QW],
                                    lhsT=KT_sb[off:off + D, pr, j * P:(j + 1) * P],
                                    rhs=QT_sb[off:off + D, pr, qb * QW:(qb + 1) * QW],
                                    start=True,
                                    stop=True,
                                )
                            pT_t = pTpool.tile([P, GW], BF16, tag="pT")
                            nc.scalar.activation(
                                out=pT_t[:, :gn * QW],
                                in_=sc_t,
                                func=mybir.ActivationFunctionType.Exp,
                                bias=nksq[:, j, h:h + 1],
                                scale=2.0 / T,
                            )
                            for qq in range(gn):
                                qb = g0 + qq
                                nc.tensor.matmul(
                                    av_ts[qb][: D + 1, :],
                                    lhsT=V_sb[:, j, h, :],
                                    rhs=pT_t[:, qq * QW:(qq + 1) * QW],
                                    start=(j == 0),
                                    stop=(j == NT - 1),
                                )
                    # drain unnormalized outputs to SBUF only; normalization
                    # is deferred past the attention loop so the PE stream
                    # stays dense (HAM re-throttles on small stalls)
                    for qb in range(NQB):
                        nc.vector.tensor_copy(
                            ou_all[: D + 1, h, qb, :], av_ts[qb][: D + 1, :]
                        )
                # normalize: out = unnorm / denom (denom = row D). Broadcast
                # the denominator row across partitions with a K=1 ones
                # matmul, then reciprocal + multiply on DVE. Normalization of
                # q-block qb immediately unblocks the out-projection of its 4
                # token tiles, so norm DVE work and out_proj matmuls
                # interleave and the PE stream stays dense through the tail.
                for qb in range(NQB):
                    for h in range(HLOC):
                        pr = h // 2
                        off = (h % 2) * D
                        # rotate bc tiles across both psum pools for depth
                        if h % 2 == 0:
                            bc_t = ps_sc.tile([P, GW], F32, tag="sc", name="bc")
                        else:
                            bc_t = ps_av.tile([P, QW], F32, tag=f"av{h}",
                                              name="bc2")
                        nc.tensor.matmul(
                            bc_t[:D, :QW],
                            lhsT=ones_col[D:D + 1, :],
                            rhs=ou_all[D:D + 1, h, qb, :],
                            start=True,
                            stop=True,
                        )
                        rb_t = dbpool.tile([D, QW], F32, tag="rb")
                        nc.vector.reciprocal(rb_t, bc_t[:D, :QW])
                        nc.vector.tensor_mul(
                            aoT[off:off + D, pr, qb * QW:(qb + 1) * QW],
                            ou_all[:D, h, qb, :],
                            rb_t,
                        )
                    for tt in range(4 * qb, 4 * qb + 4):
                        py = ps_sc.tile([P, GW], F32, tag="sc", name="py")
                        for oh in range(E // 512):
                            for s in range(2):
                                nc.tensor.matmul(
                                    py[:, oh * 512:(oh + 1) * 512],
                                    lhsT=aoT[:, s, tt * P:(tt + 1) * P],
                                    rhs=wo_sb[:, s, oh * 512:(oh + 1) * 512],
                                    start=(s == 0),
                                    stop=(s == 1),
                                )
                        y_t = ypool.tile([P, E], F32, tag="y")
                        if tt % 2 == 0:
                            nc.vector.tensor_copy(y_t, py)
                        else:
                            nc.scalar.copy(y_t, py)
                        eng = nc.sync if tt % 2 == 0 else nc.gpsimd
                        eng.dma_start(y_ds[tt][:, :], y_t)


    # run Bacc's compile passes (wait legalization, register allocation);
    # run_bass_via_pjrt binds the module without finalizing it
    nc.finalize()
    return nc


def make_in_maps(inputs, S):
    q = np.ascontiguousarray(np.asarray(inputs["query"], np.float32))
    wq = np.asarray(inputs["wq"], np.float32)
    wk = np.asarray(inputs["wk"], np.float32)
    wv = np.asarray(inputs["wv"], np.float32)
    wo = np.asarray(inputs["wo"], np.float32)
    bq = np.asarray(inputs["bq"], np.float32)
    bk = np.asarray(inputs["bk"], np.float32)
    bv = np.asarray(inputs["bv"], np.float32)
    in_maps = []
    for c in range(N_CORES):
        b = c // 4
        lo = (c % 4) * DH
        in_maps.append({
            "x": np.ascontiguousarray(q[b, :S]),
            "wq_s": np.ascontiguousarray(wq[:, lo:lo + DH]),
            "wk_s": np.ascontiguousarray(wk[:, lo:lo + DH]),
            "wv_s": np.ascontiguousarray(wv[:, lo:lo + DH]),
            "wo_s": np.ascontiguousarray(wo[lo:lo + DH, :]),
            "bq_s": np.ascontiguousarray(bq[lo:lo + DH]),
            "bk_s": np.ascontiguousarray(bk[lo:lo + DH]),
            "bv_s": np.ascontiguousarray(bv[lo:lo + DH]),
        })
    return in_maps


_prog_cache = {}


def _get_program(S, T, zq, zk, zv):
    key = (S, T, zq, zk, zv)
    if key not in _prog_cache:
        _prog_cache[key] = build_program(S, T, zq, zk, zv)
    return _prog_cache[key]


def _run(inputs, trace=False, tmpdir=None):
    S = np.asarray(inputs["query"]).shape[1]
    T = float(np.asarray(inputs["temperature"]))
    zq = not np.any(np.asarray(inputs["bq"]))
    zk = not np.any(np.asarray(inputs["bk"]))
    zv = not np.any(np.asarray(inputs["bv"]))
    nc = _get_program(S, T, zq, zk, zv)
    in_maps = make_in_maps(inputs, S)
    res = run_bass_kernel_spmd(
        nc, in_maps, list(range(N_CORES)), trace=trace, tmpdir=tmpdir
    )
    ng = S // 128
    ys = [
        np.concatenate([res.results[i][f"y{g}"] for g in range(ng)], axis=0)
        for i in range(N_CORES)
    ]
    bo = np.asarray(inputs["bo"], np.float32)
    out = np.stack([
        ys[0] + ys[1] + ys[2] + ys[3],
        ys[4] + ys[5] + ys[6] + ys[7],
    ]).astype(np.float32)
    out += bo[None, None, :]
    return out, res


def kernel(**inputs):
    out, _ = _run(inputs, trace=False)
    return out


# revision 29
# speedup vs baseline: 1.0413x; 1.0413x over previous
"""Euclidean-distance attention on 8 Trainium2 NeuronCores.

Sharding: batch (2) x head-groups (4 heads each) -> 8 cores; each core
computes Q/K/V projections for its 4 heads (column-sliced weights), a
flash-style transposed-score attention, and a partial out-projection
(row-sliced wo). Host sums the 4 partials per batch (row-parallel out_proj
reduction) and adds the output bias.

Math trick: softmax_k(-max(||q||^2+||k||^2-2qk, 0)/T) == softmax_k((2qk-||k||^2)/T)
(the ||q||^2 term is constant per row and cancels; the max() clamp never fires
because d^2 >= 0 up to rounding).  With scores computed transposed
(scT[k, q] = K @ Q^T), the per-k bias -||k||^2/T is a per-partition vector and
folds into the scalar-engine exp activation: p~ = exp(scale*scT + bias).
Normalization uses an extra all-ones column appended to V, so the softmax
denominator falls out of the same PSUM accumulation as the numerator.
"""

import sys

sys.path.insert(0, "/opt/trn_rl_repo")

import numpy as np

import concourse.bass as bass
import concourse.tile as tile
from concourse import bacc, mybir
from concourse.bass_utils import run_bass_kernel_spmd
from concourse.masks import make_identity

F32 = mybir.dt.float32
BF16 = mybir.dt.bfloat16

E = 1024          # embed dim
D = 64            # head dim
HLOC = 4          # heads per core
DH = HLOC * D     # 256: per-core projection width
P = 128
N_CORES = 8


def build_program(S, temperature, zq, zk, zv):
    """Trace the per-core program. All 8 cores run this same program on
    different input slices. zq/zk/zv: bias-is-zero flags (skip the adds)."""
    T = float(temperature)
    NT = S // P           # token tiles
    NE = E // P           # embed (contraction) tiles
    NPR = HLOC // 2       # head pairs (2)
    QW = min(512, S)      # q block width for score matmuls
    NQB = S // QW         # q blocks
    GW = 2 * QW           # exp tile width (2 q-blocks share one ACT call)

    # Bacc (not raw Bass): its compile() passes legalize multi-wait
    # instructions (move_matmul_waits_to_ldweights, generate_event_semaphores)
    # for the 1-wait-per-instruction hardware encoding.
    nc = bacc.Bacc(None)
    x_d = nc.dram_tensor("x", [S, E], F32, kind="ExternalInput")
    wq_d = nc.dram_tensor("wq_s", [E, DH], F32, kind="ExternalInput")
    wk_d = nc.dram_tensor("wk_s", [E, DH], F32, kind="ExternalInput")
    wv_d = nc.dram_tensor("wv_s", [E, DH], F32, kind="ExternalInput")
    wo_d = nc.dram_tensor("wo_s", [DH, E], F32, kind="ExternalInput")
    bq_d = nc.dram_tensor("bq_s", [DH], F32, kind="ExternalInput")
    bk_d = nc.dram_tensor("bk_s", [DH], F32, kind="ExternalInput")
    bv_d = nc.dram_tensor("bv_s", [DH], F32, kind="ExternalInput")
    # one output tensor per token tile, stored straight from PSUM: 16
    # stores spread across the 16 DMA queues (8 HWDGE + 8 SWDGE) so each
    # queue sees one store -> no same-queue ordering wait, leaving the
    # single descriptor wait slot for the RAW wait on the matmuls
    y_ds = [
        nc.dram_tensor(f"y{tt}", [P, E], F32, kind="ExternalOutput")
        for tt in range(S // P)
    ]

    def bcast_ap(ap_1d, parts):
        # [N] dram vector -> [parts, N] partition-broadcast AP
        return bass.AP(
            tensor=ap_1d.tensor, offset=ap_1d.offset, ap=[[0, parts]] + list(ap_1d.ap)
        )

    with tile.TileContext(nc) as tc:
        with tc.tile_pool(name="consts", bufs=1) as consts, \
             tc.tile_pool(name="big", bufs=1) as big, \
             tc.tile_pool(name="xbpool", bufs=3) as xbpool, \
             tc.tile_pool(name="pTpool", bufs=3) as pTpool, \
             tc.tile_pool(name="dbpool", bufs=2) as dbpool, \
             tc.tile_pool(name="xstage", bufs=1) as xstage:
            # ---- constants / weights staging ----
            ident = consts.tile([P, P], BF16)
            make_identity(nc, ident)

            # weights: casting SWDGE DMAs (f32->bf16 in flight), chunked
            # per contraction tile so the 8 SWDGE queues run in parallel
            wq_sb = consts.tile([P, NE, DH], BF16)
            wk_sb = consts.tile([P, NE, DH], BF16)
            wv_sb = consts.tile([P, NE, DH], BF16)
            wo_sb = consts.tile([P, 2, E], BF16)

            # all-ones stationary for the denominator broadcast matmul;
            # row 64 (= base_partition of the denominator row) is what's used
            ones_col = consts.tile([P, D], F32)
            nc.vector.memset(ones_col, 1.0)

            bq_col = consts.tile([P, NPR], F32)
            nc.gpsimd.dma_start(bq_col, bq_d[:].rearrange("(pr p) -> p pr", p=P))
            bk_col = consts.tile([P, NPR], F32)
            nc.gpsimd.dma_start(bk_col, bk_d[:].rearrange("(pr p) -> p pr", p=P))
            bk_bc = consts.tile([P, DH], F32)
            nc.gpsimd.dma_start(bk_bc, bcast_ap(bk_d[:], P))
            bv_bc = consts.tile([P, DH], F32)
            nc.gpsimd.dma_start(bv_bc, bcast_ap(bv_d[:], P))

            # ---- persistent big tiles ----
            xb_all = big.tile([P, NT, E], BF16)    # query, cast to bf16
            qT = big.tile([P, NE, S], BF16)        # query^T (e-major)
            QT_sb = big.tile([P, NPR, S], BF16)    # Q^T per head-pair
            KT_sb = big.tile([P, NPR, S], BF16)
            V_sb = big.tile([P, NT, HLOC, D + 1], BF16)   # V + ones column
            nksq = big.tile([P, NT, HLOC], F32)    # -||k||^2 / T
            ou_all = big.tile([P, HLOC, NQB, QW], F32)  # unnormalized attn out
            aoT = big.tile([P, NPR, S], BF16)      # normalized attn out^T

            nc.gpsimd.memset(V_sb[:, :, :, D], 1.0)

            # first x tiles via HWDGE f32 loads + DVE casts: both are much
            # lower-latency than casting SWDGE DMAs, and the first
            # transposes gate the whole projection phase
            for j in range(QW // P):
                xs = xstage.tile([P, E], F32, tag=f"xs{j}", name=f"xs{j}")
                nc.sync.dma_start(xs, x_d[j * P:(j + 1) * P, :])
                nc.vector.tensor_copy(xb_all[:, j, :], xs)
            for w_d, w_sb in ((wq_d, wq_sb), (wk_d, wk_sb), (wv_d, wv_sb)):
                wr = w_d[:].rearrange("(e p) d -> p e d", p=P)
                for e in range(NE):
                    nc.gpsimd.dma_start(w_sb[:, e, :], wr[:, e, :])
            wor = wo_d[:].rearrange("(s p) d -> p s d", p=P)
            for s2 in range(2):
                nc.gpsimd.dma_start(wo_sb[:, s2, :], wor[:, s2, :])

            # ---- phase 1+2: transpose x, projections ----
            with tc.tile_pool(name="ps_tr", bufs=2, space="PSUM") as ps_tr, \
                 tc.tile_pool(name="ps_pj", bufs=2, space="PSUM") as ps_pj, \
                 tc.tile_pool(name="ps_kv", bufs=2, space="PSUM") as ps_kv:
                for blk in range(S // QW):
                    jlo = blk * (QW // P)
                    jhi = jlo + QW // P
                    for j in range(jlo, jhi):
                        if j >= QW // P:  # first block's DMAs issued above
                            nc.gpsimd.dma_start(
                                xb_all[:, j, :], x_d[j * P:(j + 1) * P, :])
                        for e in range(NE):
                            pt = ps_tr.tile([P, P], BF16, tag="tr")
                            nc.tensor.transpose(pt, xb_all[:, j, e * P:(e + 1) * P], ident)
                            nc.vector.tensor_copy(qT[:, e, j * P:(j + 1) * P], pt)
                    bsl = slice(blk * QW, (blk + 1) * QW)
                    # Q^T and K^T per head pair over this token block
                    for pr in range(NPR):
                        psl = slice(pr * P, (pr + 1) * P)
                        for w_sb, dst, bias_col, bz in (
                            (wq_sb, QT_sb, bq_col[:, pr:pr + 1], zq),
                            (wk_sb, KT_sb, bk_col[:, pr:pr + 1], zk),
                        ):
                            pj = ps_pj.tile([P, QW], F32, tag="pj")
                            for e in range(NE):
                                nc.tensor.matmul(
                                    pj,
                                    lhsT=w_sb[:, e, psl],
                                    rhs=qT[:, e, bsl],
                                    start=(e == 0),
                                    stop=(e == NE - 1),
                                )
                            if bz:
                                nc.vector.tensor_copy(dst[:, pr, bsl], pj)
                            else:
                                # bias is per-partition (head-dim) in ^T layout
                                nc.vector.tensor_scalar_add(
                                    out=dst[:, pr, bsl], in0=pj, scalar1=bias_col
                                )
                    # V (token-major) and -||k||^2/T over this token block
                    for j in range(jlo, jhi):
                        pv = ps_kv.tile([P, DH], F32, tag="pv")
                        for e in range(NE):
                            nc.tensor.matmul(
                                pv,
                                lhsT=qT[:, e, j * P:(j + 1) * P],
                                rhs=wv_sb[:, e, :],
                                start=(e == 0),
                                stop=(e == NE - 1),
                            )
                        for h in range(HLOC):
                            if zv:
                                # scalar engine, so the attn*V matmul's waits
                                # on V and on the exp output share one sem
                                nc.scalar.copy(
                                    V_sb[:, j, h, 0:D], pv[:, h * D:(h + 1) * D]
                                )
                            else:
                                nc.vector.tensor_add(
                                    out=V_sb[:, j, h, 0:D],
                                    in0=pv[:, h * D:(h + 1) * D],
                                    in1=bv_bc[:, h * D:(h + 1) * D],
                                )
                        pk = ps_kv.tile([P, DH], F32, tag="pk")
                        for e in range(NE):
                            nc.tensor.matmul(
                                pk,
                                lhsT=qT[:, e, j * P:(j + 1) * P],
                                rhs=wk_sb[:, e, :],
                                start=(e == 0),
                                stop=(e == NE - 1),
                            )
                        sq_t = xbpool.tile([P, DH], F32, tag="sq")
                        if zk:
                            # only one non-scalar PSUM input allowed per DVE op
                            nc.vector.tensor_copy(sq_t, pk)
                        else:
                            nc.vector.tensor_add(out=sq_t, in0=pk, in1=bk_bc)
                        nc.vector.tensor_mul(sq_t, sq_t, sq_t)
                        ksqr = xbpool.tile([P, HLOC], F32, tag="ksqr")
                        nc.vector.tensor_reduce(
                            out=ksqr,
                            in_=sq_t.rearrange("p (h d) -> p h d", h=HLOC),
                            axis=mybir.AxisListType.X,
                            op=mybir.AluOpType.add,
                        )
                        nc.scalar.mul(nksq[:, j, :], ksqr, -1.0 / T)

            # ---- phase 3: attention ----
            with tc.tile_pool(name="ps_sc", bufs=2, space="PSUM") as ps_sc, \
                 tc.tile_pool(name="ps_av", bufs=1, space="PSUM") as ps_av, \
                 tc.tile_pool(name="ypool", bufs=4) as ypool:
                for h in range(HLOC):
                    pr = h // 2
                    off = (h % 2) * D
                    av_ts = [
                        ps_av.tile([P, QW], F32, tag=f"av{qb}", name=f"av{qb}")
                        for qb in range(NQB)
                    ]
                    if h == 0:
                        for qb in range(NQB):
                            nc.vector.memset(av_ts[qb], 0.0)
                    for j in range(NT):
                        for g0 in range(0, NQB, 2):
                            gn = min(2, NQB - g0)
                            sc_t = ps_sc.tile([P, gn * QW], F32, tag="sc")
                            for qq in range(gn):
                                qb = g0 + qq
                                nc.tensor.matmul(
                                    sc_t[:, qq * QW:(qq + 1) * QW],
                                    lhsT=KT_sb[off:off + D, pr, j * P:(j + 1) * P],
                                    rhs=QT_sb[off:off + D, pr, qb * QW:(qb + 1) * QW],
                                    start=True,
                                    stop=True,
                                )
                            pT_t = pTpool.tile([P, GW], BF16, tag="pT")
                            nc.scalar.activation(
                                out=pT_t[:, :gn * QW],
                                in_=sc_t,
                                func=mybir.ActivationFunctionType.Exp,
                                bias=nksq[:, j, h:h + 1],
                                scale=2.0 / T,
                            )
                            for qq in range(gn):
                                qb = g0 + qq
                                nc.tensor.matmul(
                                    av_ts[qb][: D + 1, :],
                                    lhsT=V_sb[:, j, h, :],
                                    rhs=pT_t[:, qq * QW:(qq + 1) * QW],
                                    start=(j == 0),
                                    stop=(j == NT - 1),
                                )
                    # drain unnormalized outputs to SBUF only; normalization
                    # is deferred past the attention loop so the PE stream
                    # stays dense (HAM re-throttles on small stalls)
                    for qb in range(NQB):
                        nc.vector.tensor_copy(
                            ou_all[: D + 1, h, qb, :], av_ts[qb][: D + 1, :]
                        )
                # normalize: out = unnorm / denom (denom = row D). Broadcast
                # the denominator row across partitions with a K=1 ones
                # matmul, then reciprocal + multiply on DVE. Normalization of
                # q-block qb immediately unblocks the out-projection of its 4
                # token tiles, so norm DVE work and out_proj matmuls
                # interleave and the PE stream stays dense through the tail.
                for qb in range(NQB):
                    for h in range(HLOC):
                        pr = h // 2
                        off = (h % 2) * D
                        # rotate bc tiles across both psum pools for depth
                        if h % 2 == 0:
                            bc_t = ps_sc.tile([P, GW], F32, tag="sc", name="bc")
                        else:
                            bc_t = ps_av.tile([P, QW], F32, tag=f"av{h}",
                                              name="bc2")
                        nc.tensor.matmul(
                            bc_t[:D, :QW],
                            lhsT=ones_col[D:D + 1, :],
                            rhs=ou_all[D:D + 1, h, qb, :],
                            start=True,
                            stop=True,
                        )
                        rb_t = dbpool.tile([D, QW], F32, tag="rb")
                        nc.vector.reciprocal(rb_t, bc_t[:D, :QW])
                        nc.vector.tensor_mul(
                            aoT[off:off + D, pr, qb * QW:(qb + 1) * QW],
                            ou_all[:D, h, qb, :],
                            rb_t,
                        )
                    for tt in range(4 * qb, 4 * qb + 4):
                        py = ps_sc.tile([P, GW], F32, tag="sc", name="py")
                        for oh in range(E // 512):
                            for s in range(2):
                                nc.tensor.matmul(
                                    py[:, oh * 512:(oh + 1) * 512],
                                    lhsT=aoT[:, s, tt * P:(tt + 1) * P],
                                    rhs=wo_sb[:, s, oh * 512:(oh + 1) * 512],
                                    start=(s == 0),
                                    stop=(s == 1),
                                )
                        y_t = ypool.tile([P, E], F32, tag="y")
                        if tt % 2 == 0:
                            nc.vector.tensor_copy(y_t, py)
                        else:
                            nc.scalar.copy(y_t, py)
                        eng = nc.sync if tt % 2 == 0 else nc.gpsimd
                        eng.dma_start(y_ds[tt][:, :], y_t)


    # run Bacc's compile passes (wait legalization, register allocation);
    # run_bass_via_pjrt binds the module without finalizing it
    nc.finalize()
    return nc


def make_in_maps(inputs, S):
    q = np.ascontiguousarray(np.asarray(inputs["query"], np.float32))
    wq = np.asarray(inputs["wq"], np.float32)
    wk = np.asarray(inputs["wk"], np.float32)
    wv = np.asarray(inputs["wv"], np.float32)
    wo = np.asarray(inputs["wo"], np.float32)
    bq = np.asarray(inputs["bq"], np.float32)
    bk = np.asarray(inputs["bk"], np.float32)
    bv = np.asarray(inputs["bv"], np.float32)
    in_maps = []
    for c in range(N_CORES):
        b = c // 4
        lo = (c % 4) * DH
        in_maps.append({
            "x": np.ascontiguousarray(q[b, :S]),
            "wq_s": np.ascontiguousarray(wq[:, lo:lo + DH]),
            "wk_s": np.ascontiguousarray(wk[:, lo:lo + DH]),
            "wv_s": np.ascontiguousarray(wv[:, lo:lo + DH]),
            "wo_s": np.ascontiguousarray(wo[lo:lo + DH, :]),
            "bq_s": np.ascontiguousarray(bq[lo:lo + DH]),
            "bk_s": np.ascontiguousarray(bk[lo:lo + DH]),
            "bv_s": np.ascontiguousarray(bv[lo:lo + DH]),
        })
    return in_maps


_prog_cache = {}


def _get_program(S, T, zq, zk, zv):
    key = (S, T, zq, zk, zv)
    if key not in _prog_cache:
        _prog_cache[key] = build_program(S, T, zq, zk, zv)
    return _prog_cache[key]


def _run(inputs, trace=False, tmpdir=None):
    S = np.asarray(inputs["query"]).shape[1]
    T = float(np.asarray(inputs["temperature"]))
    zq = not np.any(np.asarray(inputs["bq"]))
    zk = not np.any(np.asarray(inputs["bk"]))
    zv = not np.any(np.asarray(inputs["bv"]))
    nc = _get_program(S, T, zq, zk, zv)
    in_maps = make_in_maps(inputs, S)
    res = run_bass_kernel_spmd(
        nc, in_maps, list(range(N_CORES)), trace=trace, tmpdir=tmpdir
    )
    ng = S // 128
    ys = [
        np.concatenate([res.results[i][f"y{g}"] for g in range(ng)], axis=0)
        for i in range(N_CORES)
    ]
    bo = np.asarray(inputs["bo"], np.float32)
    out = np.stack([
        ys[0] + ys[1] + ys[2] + ys[3],
        ys[4] + ys[5] + ys[6] + ys[7],
    ]).astype(np.float32)
    out += bo[None, None, :]
    return out, res


def kernel(**inputs):
    out, _ = _run(inputs, trace=False)
    return out


# revision 30
# speedup vs baseline: 1.0470x; 1.0055x over previous
"""Euclidean-distance attention on 8 Trainium2 NeuronCores.

Sharding: batch (2) x head-groups (4 heads each) -> 8 cores; each core
computes Q/K/V projections for its 4 heads (column-sliced weights), a
flash-style transposed-score attention, and a partial out-projection
(row-sliced wo). Host sums the 4 partials per batch (row-parallel out_proj
reduction) and adds the output bias.

Math trick: softmax_k(-max(||q||^2+||k||^2-2qk, 0)/T) == softmax_k((2qk-||k||^2)/T)
(the ||q||^2 term is constant per row and cancels; the max() clamp never fires
because d^2 >= 0 up to rounding).  With scores computed transposed
(scT[k, q] = K @ Q^T), the per-k bias -||k||^2/T is a per-partition vector and
folds into the scalar-engine exp activation: p~ = exp(scale*scT + bias).
Normalization uses an extra all-ones column appended to V, so the softmax
denominator falls out of the same PSUM accumulation as the numerator.
"""

import sys

sys.path.insert(0, "/opt/trn_rl_repo")

import numpy as np

import concourse.bass as bass
import concourse.tile as tile
from concourse import bacc, mybir
from concourse.bass_utils import run_bass_kernel_spmd
from concourse.masks import make_identity

F32 = mybir.dt.float32
BF16 = mybir.dt.bfloat16

E = 1024          # embed dim
D = 64            # head dim
HLOC = 4          # heads per core
DH = HLOC * D     # 256: per-core projection width
P = 128
N_CORES = 8


def build_program(S, temperature, zq, zk, zv):
    """Trace the per-core program. All 8 cores run this same program on
    different input slices. zq/zk/zv: bias-is-zero flags (skip the adds)."""
    T = float(temperature)
    NT = S // P           # token tiles
    NE = E // P           # embed (contraction) tiles
    NPR = HLOC // 2       # head pairs (2)
    QW = min(512, S)      # q block width for score matmuls
    NQB = S // QW         # q blocks
    GW = 2 * QW           # exp tile width (2 q-blocks share one ACT call)

    # Bacc (not raw Bass): its compile() passes legalize multi-wait
    # instructions (move_matmul_waits_to_ldweights, generate_event_semaphores)
    # for the 1-wait-per-instruction hardware encoding.
    nc = bacc.Bacc(None)
    x_d = nc.dram_tensor("x", [S, E], F32, kind="ExternalInput")
    wq_d = nc.dram_tensor("wq_s", [E, DH], F32, kind="ExternalInput")
    wk_d = nc.dram_tensor("wk_s", [E, DH], F32, kind="ExternalInput")
    wv_d = nc.dram_tensor("wv_s", [E, DH], F32, kind="ExternalInput")
    wo_d = nc.dram_tensor("wo_s", [DH, E], F32, kind="ExternalInput")
    bq_d = nc.dram_tensor("bq_s", [DH], F32, kind="ExternalInput")
    bk_d = nc.dram_tensor("bk_s", [DH], F32, kind="ExternalInput")
    bv_d = nc.dram_tensor("bv_s", [DH], F32, kind="ExternalInput")
    # one output tensor per token tile, stored straight from PSUM: 16
    # stores spread across the 16 DMA queues (8 HWDGE + 8 SWDGE) so each
    # queue sees one store -> no same-queue ordering wait, leaving the
    # single descriptor wait slot for the RAW wait on the matmuls
    y_ds = [
        nc.dram_tensor(f"y{tt}", [P, E], F32, kind="ExternalOutput")
        for tt in range(S // P)
    ]

    def bcast_ap(ap_1d, parts):
        # [N] dram vector -> [parts, N] partition-broadcast AP
        return bass.AP(
            tensor=ap_1d.tensor, offset=ap_1d.offset, ap=[[0, parts]] + list(ap_1d.ap)
        )

    with tile.TileContext(nc) as tc:
        with tc.tile_pool(name="consts", bufs=1) as consts, \
             tc.tile_pool(name="big", bufs=1) as big, \
             tc.tile_pool(name="xbpool", bufs=3) as xbpool, \
             tc.tile_pool(name="pTpool", bufs=3) as pTpool, \
             tc.tile_pool(name="dbpool", bufs=4) as dbpool, \
             tc.tile_pool(name="xstage", bufs=1) as xstage:
            # ---- constants / weights staging ----
            ident = consts.tile([P, P], BF16)
            make_identity(nc, ident)

            # weights: casting SWDGE DMAs (f32->bf16 in flight), chunked
            # per contraction tile so the 8 SWDGE queues run in parallel
            wq_sb = consts.tile([P, NE, DH], BF16)
            wk_sb = consts.tile([P, NE, DH], BF16)
            wv_sb = consts.tile([P, NE, DH], BF16)
            wo_sb = consts.tile([P, 2, E], BF16)

            # all-ones stationary for the denominator broadcast matmul;
            # row 64 (= base_partition of the denominator row) is what's used
            ones_col = consts.tile([P, D], F32)
            nc.vector.memset(ones_col, 1.0)

            bq_col = consts.tile([P, NPR], F32)
            nc.gpsimd.dma_start(bq_col, bq_d[:].rearrange("(pr p) -> p pr", p=P))
            bk_col = consts.tile([P, NPR], F32)
            nc.gpsimd.dma_start(bk_col, bk_d[:].rearrange("(pr p) -> p pr", p=P))
            bk_bc = consts.tile([P, DH], F32)
            nc.gpsimd.dma_start(bk_bc, bcast_ap(bk_d[:], P))
            bv_bc = consts.tile([P, DH], F32)
            nc.gpsimd.dma_start(bv_bc, bcast_ap(bv_d[:], P))

            # ---- persistent big tiles ----
            xb_all = big.tile([P, NT, E], BF16)    # query, cast to bf16
            qT = big.tile([P, NE, S], BF16)        # query^T (e-major)
            QT_sb = big.tile([P, NPR, S], BF16)    # Q^T per head-pair
            KT_sb = big.tile([P, NPR, S], BF16)
            V_sb = big.tile([P, NT, HLOC, D + 1], BF16)   # V + ones column
            nksq = big.tile([P, NT, HLOC], F32)    # -||k||^2 / T
            ou_all = big.tile([P, HLOC, NQB, QW], F32)  # unnormalized attn out
            aoT = big.tile([P, NPR, S], BF16)      # normalized attn out^T

            nc.gpsimd.memset(V_sb[:, :, :, D], 1.0)

            # first x tiles via HWDGE f32 loads + DVE casts: both are much
            # lower-latency than casting SWDGE DMAs, and the first
            # transposes gate the whole projection phase
            for j in range(QW // P):
                xs = xstage.tile([P, E], F32, tag=f"xs{j}", name=f"xs{j}")
                nc.sync.dma_start(xs, x_d[j * P:(j + 1) * P, :])
                nc.vector.tensor_copy(xb_all[:, j, :], xs)
            for w_d, w_sb in ((wq_d, wq_sb), (wk_d, wk_sb), (wv_d, wv_sb)):
                wr = w_d[:].rearrange("(e p) d -> p e d", p=P)
                for e in range(NE):
                    nc.gpsimd.dma_start(w_sb[:, e, :], wr[:, e, :])
            wor = wo_d[:].rearrange("(s p) d -> p s d", p=P)
            for s2 in range(2):
                nc.gpsimd.dma_start(wo_sb[:, s2, :], wor[:, s2, :])

            # ---- phase 1+2: transpose x, projections ----
            with tc.tile_pool(name="ps_tr", bufs=2, space="PSUM") as ps_tr, \
                 tc.tile_pool(name="ps_pj", bufs=2, space="PSUM") as ps_pj, \
                 tc.tile_pool(name="ps_kv", bufs=2, space="PSUM") as ps_kv:
                for blk in range(S // QW):
                    jlo = blk * (QW // P)
                    jhi = jlo + QW // P
                    for j in range(jlo, jhi):
                        if j >= QW // P:  # first block's DMAs issued above
                            nc.gpsimd.dma_start(
                                xb_all[:, j, :], x_d[j * P:(j + 1) * P, :])
                        for e in range(NE):
                            pt = ps_tr.tile([P, P], BF16, tag="tr")
                            nc.tensor.transpose(pt, xb_all[:, j, e * P:(e + 1) * P], ident)
                            nc.vector.tensor_copy(qT[:, e, j * P:(j + 1) * P], pt)
                    bsl = slice(blk * QW, (blk + 1) * QW)
                    # Q^T and K^T per head pair over this token block
                    for pr in range(NPR):
                        psl = slice(pr * P, (pr + 1) * P)
                        for w_sb, dst, bias_col, bz in (
                            (wq_sb, QT_sb, bq_col[:, pr:pr + 1], zq),
                            (wk_sb, KT_sb, bk_col[:, pr:pr + 1], zk),
                        ):
                            pj = ps_pj.tile([P, QW], F32, tag="pj")
                            for e in range(NE):
                                nc.tensor.matmul(
                                    pj,
                                    lhsT=w_sb[:, e, psl],
                                    rhs=qT[:, e, bsl],
                                    start=(e == 0),
                                    stop=(e == NE - 1),
                                )
                            if bz:
                                nc.vector.tensor_copy(dst[:, pr, bsl], pj)
                            else:
                                # bias is per-partition (head-dim) in ^T layout
                                nc.vector.tensor_scalar_add(
                                    out=dst[:, pr, bsl], in0=pj, scalar1=bias_col
                                )
                    # V (token-major) and -||k||^2/T over this token block
                    for j in range(jlo, jhi):
                        pv = ps_kv.tile([P, DH], F32, tag="pv")
                        for e in range(NE):
                            nc.tensor.matmul(
                                pv,
                                lhsT=qT[:, e, j * P:(j + 1) * P],
                                rhs=wv_sb[:, e, :],
                                start=(e == 0),
                                stop=(e == NE - 1),
                            )
                        for h in range(HLOC):
                            if zv:
                                # scalar engine, so the attn*V matmul's waits
                                # on V and on the exp output share one sem
                                nc.scalar.copy(
                                    V_sb[:, j, h, 0:D], pv[:, h * D:(h + 1) * D]
                                )
                            else:
                                nc.vector.tensor_add(
                                    out=V_sb[:, j, h, 0:D],
                                    in0=pv[:, h * D:(h + 1) * D],
                                    in1=bv_bc[:, h * D:(h + 1) * D],
                                )
                        pk = ps_kv.tile([P, DH], F32, tag="pk")
                        for e in range(NE):
                            nc.tensor.matmul(
                                pk,
                                lhsT=qT[:, e, j * P:(j + 1) * P],
                                rhs=wk_sb[:, e, :],
                                start=(e == 0),
                                stop=(e == NE - 1),
                            )
                        sq_t = xbpool.tile([P, DH], F32, tag="sq")
                        if zk:
                            # only one non-scalar PSUM input allowed per DVE op
                            nc.vector.tensor_copy(sq_t, pk)
                        else:
                            nc.vector.tensor_add(out=sq_t, in0=pk, in1=bk_bc)
                        nc.vector.tensor_mul(sq_t, sq_t, sq_t)
                        ksqr = xbpool.tile([P, HLOC], F32, tag="ksqr")
                        nc.vector.tensor_reduce(
                            out=ksqr,
                            in_=sq_t.rearrange("p (h d) -> p h d", h=HLOC),
                            axis=mybir.AxisListType.X,
                            op=mybir.AluOpType.add,
                        )
                        nc.scalar.mul(nksq[:, j, :], ksqr, -1.0 / T)

            # ---- phase 3: attention ----
            with tc.tile_pool(name="ps_sc", bufs=2, space="PSUM") as ps_sc, \
                 tc.tile_pool(name="ps_av", bufs=1, space="PSUM") as ps_av, \
                 tc.tile_pool(name="ypool", bufs=4) as ypool:
                for h in range(HLOC):
                    pr = h // 2
                    off = (h % 2) * D
                    av_ts = [
                        ps_av.tile([P, QW], F32, tag=f"av{qb}", name=f"av{qb}")
                        for qb in range(NQB)
                    ]
                    if h == 0:
                        for qb in range(NQB):
                            nc.vector.memset(av_ts[qb], 0.0)
                    for j in range(NT):
                        for g0 in range(0, NQB, 2):
                            gn = min(2, NQB - g0)
                            sc_t = ps_sc.tile([P, gn * QW], F32, tag="sc")
                            for qq in range(gn):
                                qb = g0 + qq
                                nc.tensor.matmul(
                                    sc_t[:, qq * QW:(qq + 1) * QW],
                                    lhsT=KT_sb[off:off + D, pr, j * P:(j + 1) * P],
                                    rhs=QT_sb[off:off + D, pr, qb * QW:(qb + 1) * QW],
                                    start=True,
                                    stop=True,
                                )
                            pT_t = pTpool.tile([P, GW], BF16, tag="pT")
                            nc.scalar.activation(
                                out=pT_t[:, :gn * QW],
                                in_=sc_t,
                                func=mybir.ActivationFunctionType.Exp,
                                bias=nksq[:, j, h:h + 1],
                                scale=2.0 / T,
                            )
                            for qq in range(gn):
                                qb = g0 + qq
                                nc.tensor.matmul(
                                    av_ts[qb][: D + 1, :],
                                    lhsT=V_sb[:, j, h, :],
                                    rhs=pT_t[:, qq * QW:(qq + 1) * QW],
                                    start=(j == 0),
                                    stop=(j == NT - 1),
                                )
                    # drain unnormalized outputs to SBUF only; normalization
                    # is deferred past the attention loop so the PE stream
                    # stays dense (HAM re-throttles on small stalls)
                    for qb in range(NQB):
                        nc.vector.tensor_copy(
                            ou_all[: D + 1, h, qb, :], av_ts[qb][: D + 1, :]
                        )
                # normalize: out = unnorm / denom (denom = row D). Broadcast
                # the denominator row across partitions with a K=1 ones
                # matmul, then reciprocal + multiply on DVE. Normalization of
                # q-block qb immediately unblocks the out-projection of its 4
                # token tiles, so norm DVE work and out_proj matmuls
                # interleave and the PE stream stays dense through the tail.
                for qb in range(NQB):
                    for h in range(HLOC):
                        pr = h // 2
                        off = (h % 2) * D
                        # rotate bc tiles across both psum pools for depth
                        if h % 2 == 0:
                            bc_t = ps_sc.tile([P, GW], F32, tag="sc", name="bc")
                        else:
                            bc_t = ps_av.tile([P, QW], F32, tag=f"av{h}",
                                              name="bc2")
                        nc.tensor.matmul(
                            bc_t[:D, :QW],
                            lhsT=ones_col[D:D + 1, :],
                            rhs=ou_all[D:D + 1, h, qb, :],
                            start=True,
                            stop=True,
                        )
                        rb_t = dbpool.tile([D, QW], F32, tag="rb")
                        nc.vector.reciprocal(rb_t, bc_t[:D, :QW])
                        nc.vector.tensor_mul(
                            aoT[off:off + D, pr, qb * QW:(qb + 1) * QW],
                            ou_all[:D, h, qb, :],
                            rb_t,
                        )
                    for tt in range(4 * qb, 4 * qb + 4):
                        py = ps_sc.tile([P, GW], F32, tag="sc", name="py")
                        for oh in range(E // 512):
                            for s in range(2):
                                nc.tensor.matmul(
                                    py[:, oh * 512:(oh + 1) * 512],
                                    lhsT=aoT[:, s, tt * P:(tt + 1) * P],
                                    rhs=wo_sb[:, s, oh * 512:(oh + 1) * 512],
                                    start=(s == 0),
                                    stop=(s == 1),
                                )
                        y_t = ypool.tile([P, E], F32, tag="y")
                        if tt % 2 == 0:
                            nc.vector.tensor_copy(y_t, py)
                        else:
                            nc.scalar.copy(y_t, py)
                        eng = nc.sync if tt % 2 == 0 else nc.gpsimd
                        eng.dma_start(y_ds[tt][:, :], y_t)


    # run Bacc's compile passes (wait legalization, register allocation);
    # run_bass_via_pjrt binds the module without finalizing it
    nc.finalize()
    return nc


def make_in_maps(inputs, S):
    q = np.ascontiguousarray(np.asarray(inputs["query"], np.float32))
    wq = np.asarray(inputs["wq"], np.float32)
    wk = np.asarray(inputs["wk"], np.float32)
    wv = np.asarray(inputs["wv"], np.float32)
    wo = np.asarray(inputs["wo"], np.float32)
    bq = np.asarray(inputs["bq"], np.float32)
    bk = np.asarray(inputs["bk"], np.float32)
    bv = np.asarray(inputs["bv"], np.float32)
    in_maps = []
    for c in range(N_CORES):
        b = c // 4
        lo = (c % 4) * DH
        in_maps.append({
            "x": np.ascontiguousarray(q[b, :S]),
            "wq_s": np.ascontiguousarray(wq[:, lo:lo + DH]),
            "wk_s": np.ascontiguousarray(wk[:, lo:lo + DH]),
            "wv_s": np.ascontiguousarray(wv[:, lo:lo + DH]),
            "wo_s": np.ascontiguousarray(wo[lo:lo + DH, :]),
            "bq_s": np.ascontiguousarray(bq[lo:lo + DH]),
            "bk_s": np.ascontiguousarray(bk[lo:lo + DH]),
            "bv_s": np.ascontiguousarray(bv[lo:lo + DH]),
        })
    return in_maps


_prog_cache = {}


def _get_program(S, T, zq, zk, zv):
    key = (S, T, zq, zk, zv)
    if key not in _prog_cache:
        _prog_cache[key] = build_program(S, T, zq, zk, zv)
    return _prog_cache[key]


def _run(inputs, trace=False, tmpdir=None):
    S = np.asarray(inputs["query"]).shape[1]
    T = float(np.asarray(inputs["temperature"]))
    zq = not np.any(np.asarray(inputs["bq"]))
    zk = not np.any(np.asarray(inputs["bk"]))
    zv = not np.any(np.asarray(inputs["bv"]))
    nc = _get_program(S, T, zq, zk, zv)
    in_maps = make_in_maps(inputs, S)
    res = run_bass_kernel_spmd(
        nc, in_maps, list(range(N_CORES)), trace=trace, tmpdir=tmpdir
    )
    ng = S // 128
    ys = [
        np.concatenate([res.results[i][f"y{g}"] for g in range(ng)], axis=0)
        for i in range(N_CORES)
    ]
    bo = np.asarray(inputs["bo"], np.float32)
    out = np.stack([
        ys[0] + ys[1] + ys[2] + ys[3],
        ys[4] + ys[5] + ys[6] + ys[7],
    ]).astype(np.float32)
    out += bo[None, None, :]
    return out, res


def kernel(**inputs):
    out, _ = _run(inputs, trace=False)
    return out
